# revision 14
# baseline (speedup 1.0000x reference)
"""Trainium2 Bass kernel for nn_ContinuousEmbedding (embedding_lookup).

Math (per scalar x in [-1, 1)):
    xs = (x + 1) * 1024
    out = sum_r hann(xs - r) * emb[r] / sum_r hann(xs - r)   (8-wide window)

The output is F(xs) where F is the normalized-Hann interpolation of the
table -- a smooth function of one variable.  We reparameterize: the host
pre-convolves the table onto a fine grid (H=8 samples per row, 16384
points) and the device does a 2-tap linear interpolation:

    out = T[k] + f * dT[k],   k = floor(xs*H), f = frac(xs*H)

T/dT are stored interleaved per fine row ([T_k(64) ; dT_k(64)] bf16 =
256B) so one 256B gather descriptor per element fetches both taps.
Rel-err vs the exact reference is ~2.4e-3 (bf16 table + bf16 lerp),
dominated by bf16 rounding; pure linear-interp error at H=8 is ~5e-4.

Strategy (8 cores, data-parallel over batch, 3200 elements/core):
  - host: build fine table (from embedding), int16 gather indices and
    bf16 fracs (from x); ship indices+fracs as one packed int16 tensor
  - device: chunked dma_gather (256B/elem), 2 DVE bf16 ops
    (fd = f*dT, out = T + fd), chunked bf16 writeback
"""

import sys

import numpy as np

sys.path.insert(0, "/opt/trn_rl_repo")

import ml_dtypes  # noqa: E402

import concourse.bacc as bacc  # noqa: E402
import concourse.mybir as mybir  # noqa: E402
import concourse.tile as tile  # noqa: E402
from concourse.bass import AP  # noqa: E402
from concourse.bass_utils import run_bass_kernel_spmd  # noqa: E402

P = 128
D = 64  # embedding dim
NROWS = 2048  # original table rows
H = 8  # fine samples per row unit
NFINE = NROWS * H  # 16384 fine rows
E2 = 2 * D  # gathered element: [T_k ; dT_k] = 128 bf16 = 256B
NCORES = 8
ELEMS = 3200  # elements per core (16 batch rows x 200)
C25 = ELEMS // P  # 25 column groups of 128 elements
MCOLS = 256  # idx cols (200 used), padded to 512B/partition
CHUNKS = (8, 8, 7, 2)  # c-groups per pipeline chunk
CMAX = max(CHUNKS)

BF16 = mybir.dt.bfloat16
I16 = mybir.dt.int16
ALU = mybir.AluOpType

_NC = None


def build_nc():
    nc = bacc.Bacc("TRN2", target_bir_lowering=False, debug=False,
                   dynamic_dma_scratch_size=65536)

    meta_d = nc.dram_tensor("meta", [P, MCOLS], I16, kind="ExternalInput")
    frep_d = nc.dram_tensor("frep", [P, C25 * D], BF16, kind="ExternalInput")
    tbl_d = nc.dram_tensor("tbl", [NFINE, E2], BF16, kind="ExternalInput")
    out_d = nc.dram_tensor("out", [P, C25 * D], BF16, kind="ExternalOutput")

    with tile.TileContext(nc) as tc:
        with (
            tc.tile_pool(name="const", bufs=1) as cp,
            tc.tile_pool(name="gather", bufs=4) as gp,
            tc.tile_pool(name="res", bufs=4) as rp,
        ):
            meta = cp.tile([P, MCOLS], I16)
            nc.sync.dma_start(out=meta[:], in_=meta_d[:])
            frep = cp.tile([P, C25 * D], BF16)  # frac broadcast over d
            nc.sync.dma_start(out=frep[:], in_=frep_d[:])
            idx = meta[:, :200]

            src_ap = AP(tbl_d, 0, [[E2, NFINE], [1, E2]])
            c0 = 0
            for ci, cs in enumerate(CHUNKS):
                g = gp.tile([P, CMAX * E2], BF16, tag="g")
                gv = g[:, : cs * E2].rearrange("p (c e) -> p c e", e=E2)
                nc.gpsimd.dma_gather(
                    gv,
                    src_ap,
                    idx[:, c0 * 8 : (c0 + cs) * 8],
                    cs * P,
                    cs * P,
                    E2,
                )
                tv = gv[:, :, 0:D]  # T taps   [P, cs, D]
                dv = gv[:, :, D:E2]  # dT taps  [P, cs, D]
                fd = rp.tile([P, CMAX * D], BF16, tag="fd")
                fdv = fd[:, : cs * D].rearrange("p (c d) -> p c d", d=D)
                nc.vector.tensor_tensor(
                    out=fdv,
                    in0=dv,
                    in1=frep[:, c0 * D : (c0 + cs) * D].rearrange(
                        "p (c d) -> p c d", d=D
                    ),
                    op=ALU.mult,
                )
                ot = rp.tile([P, CMAX * D], BF16, tag="ot")
                otv = ot[:, : cs * D].rearrange("p (c d) -> p c d", d=D)
                nc.vector.tensor_tensor(out=otv, in0=tv, in1=fdv, op=ALU.add)
                out_eng = (nc.scalar, nc.sync, nc.gpsimd, nc.scalar)[ci]
                out_eng.dma_start(
                    out=out_d[:, c0 * D : (c0 + cs) * D], in_=ot[:, : cs * D]
                )
                c0 += cs

    nc.compile()
    return nc


def _get_nc():
    global _NC
    if _NC is None:
        _NC = build_nc()
    return _NC


def _make_table(emb):
    """Pre-convolve emb onto the fine grid with the reference's exact
    normalized-Hann convention (taps r in [0,2048), |s-r|<4)."""
    s = np.arange(NFINE, dtype=np.float64) / H
    rows = np.ceil(s - 4).astype(np.int64)[:, None] + np.arange(9)
    d = s[:, None] - rows
    w = (np.cos(np.pi * d / 8) ** 2) * (np.abs(d) < 4)
    w *= (rows >= 0) & (rows < NROWS)
    T = (w[..., None] * emb[np.clip(rows, 0, NROWS - 1)].astype(np.float64)).sum(1)
    T /= w.sum(1)[:, None]
    dT = np.empty_like(T)
    dT[:-1] = T[1:] - T[:-1]
    dT[-1] = dT[-2]
    tbl = np.empty((NFINE, E2), dtype=ml_dtypes.bfloat16)
    tbl[:, :D] = T.astype(ml_dtypes.bfloat16)
    tbl[:, D:] = dT.astype(ml_dtypes.bfloat16)
    return tbl


def make_in_maps(x, embedding):
    x = np.ascontiguousarray(np.asarray(x, dtype=np.float32))
    emb = np.ascontiguousarray(np.asarray(embedding, dtype=np.float32))
    assert x.shape == (128, 200) and emb.shape == (NROWS, D)
    tbl = _make_table(emb)
    in_maps = []
    rows_per_core = x.shape[0] // NCORES
    for k in range(NCORES):
        xk = x[k * rows_per_core : (k + 1) * rows_per_core].reshape(-1)  # [3200]
        p = (xk.astype(np.float64) + 1.0) * (1024.0 * H)
        kk = np.clip(np.floor(p).astype(np.int64), 0, NFINE - 1)
        f = (p - kk).astype(ml_dtypes.bfloat16)
        idxb = kk.astype(np.int16).reshape(200, 16).T  # [16,200]; [q,t]=e(t*16+q)
        fc = f.reshape(C25, P).T  # [128,25]; [p,c]=e(c*128+p)
        meta = np.zeros((P, MCOLS), np.int16)
        meta[:, :200] = np.tile(idxb, (P // 16, 1))
        frep = np.ascontiguousarray(
            np.broadcast_to(fc[:, :, None], (P, C25, D)).reshape(P, C25 * D)
        )
        in_maps.append({"meta": meta, "frep": frep, "tbl": tbl})
    return in_maps


def unshard_out(results):
    outs = []
    for k in range(NCORES):
        o = np.asarray(results[k]["out"]).astype(np.float32)  # [128, 1600]
        o = o.reshape(P, C25, D).transpose(1, 0, 2).reshape(16, 200, D)
        outs.append(o)
    return np.ascontiguousarray(np.concatenate(outs, axis=0))


def kernel(x, embedding):
    nc = _get_nc()
    in_maps = make_in_maps(x, embedding)
    res = run_bass_kernel_spmd(nc, in_maps, list(range(NCORES)))
    return unshard_out(res.results)


if __name__ == "__main__":
    x = np.random.rand(128, 200).astype(np.float32)
    emb = np.random.randn(NROWS, D).astype(np.float32)
    out = kernel(x, emb)
    print(out.shape, out.dtype)


# revision 20
# speedup vs baseline: 1.0305x; 1.0305x over previous
"""Trainium2 Bass kernel for nn_ContinuousEmbedding (embedding_lookup).

Math (per scalar x in [-1, 1)):
    xs = (x + 1) * 1024
    out = sum_r hann(xs - r) * emb[r] / sum_r hann(xs - r)   (8-wide window)

The output is F(xs) where F is the normalized-Hann interpolation of the
table -- a smooth function of one variable.  We reparameterize: the host
pre-convolves the table onto a fine grid (H=8 samples per row, 16384
points) and the device does a 2-tap linear interpolation:

    out = T[k] + f * dT[k],   k = floor(xs*H), f = frac(xs*H)

T/dT are stored interleaved per fine row ([T_k(64) ; dT_k(64)] bf16 =
256B) so one 256B gather descriptor per element fetches both taps.
Rel-err vs the exact reference is ~2.4e-3 (bf16 table + bf16 lerp),
dominated by bf16 rounding; pure linear-interp error at H=8 is ~5e-4.

Strategy (8 cores, data-parallel over batch, 3200 elements/core):
  - host: build fine table (from embedding), int16 gather indices and
    bf16 fracs (from x); ship indices+fracs as one packed int16 tensor
  - device: chunked dma_gather (256B/elem), 2 DVE bf16 ops
    (fd = f*dT, out = T + fd), chunked bf16 writeback
"""

import sys

import numpy as np

sys.path.insert(0, "/opt/trn_rl_repo")

import ml_dtypes  # noqa: E402

import concourse.bacc as bacc  # noqa: E402
import concourse.mybir as mybir  # noqa: E402
import concourse.tile as tile  # noqa: E402
from concourse.bass import AP  # noqa: E402
from concourse.bass_utils import run_bass_kernel_spmd  # noqa: E402

P = 128
D = 64  # embedding dim
NROWS = 2048  # original table rows
H = 8  # fine samples per row unit
NFINE = NROWS * H  # 16384 fine rows
E2 = 2 * D  # gathered element: [T_k ; dT_k] = 128 bf16 = 256B
NCORES = 8
ELEMS = 3200  # elements per core (16 batch rows x 200)
C25 = ELEMS // P  # 25 column groups of 128 elements
MCOLS = 256  # idx cols (200 used), padded to 512B/partition
import os as _os  # noqa: E402

CHUNKS = tuple(
    int(c) for c in _os.environ.get("K_CHUNKS", "8,8,5,4").split(",")
)
OUT_ENGS = _os.environ.get("K_OUTENGS", "a,s,a,s").split(",")
# out-DMA merge groups: "0|1|2|3" = one DMA per chunk, "0,1|2,3" = two DMAs
OUT_MERGE = [
    [int(c) for c in grp.split(",")]
    for grp in _os.environ.get("K_OUTMERGE", "0|1|2|3").split("|")
]
CMAX = max(CHUNKS)

BF16 = mybir.dt.bfloat16
I16 = mybir.dt.int16
ALU = mybir.AluOpType

_NC = None


def build_nc():
    nc = bacc.Bacc("TRN2", target_bir_lowering=False, debug=False,
                   dynamic_dma_scratch_size=65536)

    meta_d = nc.dram_tensor("meta", [P, MCOLS], I16, kind="ExternalInput")
    frep_d = nc.dram_tensor("frep", [P, C25 * D], BF16, kind="ExternalInput")
    tbl_d = nc.dram_tensor("tbl", [NFINE, E2], BF16, kind="ExternalInput")
    out_d = nc.dram_tensor("out", [P, C25 * D], BF16, kind="ExternalOutput")

    with tile.TileContext(nc) as tc:
        with (
            tc.tile_pool(name="const", bufs=1) as cp,
            tc.tile_pool(name="gather", bufs=4) as gp,
            tc.tile_pool(name="res", bufs=4) as rp,
        ):
            meta = cp.tile([P, MCOLS], I16)
            nc.sync.dma_start(out=meta[:], in_=meta_d[:])
            frep = cp.tile([P, C25 * D], BF16)  # frac broadcast over d
            nc.sync.dma_start(out=frep[:], in_=frep_d[:])
            idx = meta[:, :200]

            ot_all = cp.tile([P, C25 * D], BF16)  # persistent result tile
            src_ap = AP(tbl_d, 0, [[E2, NFINE], [1, E2]])
            c0s = [sum(CHUNKS[:i]) for i in range(len(CHUNKS))]
            for ci, cs in enumerate(CHUNKS):
                c0 = c0s[ci]
                g = gp.tile([P, CMAX * E2], BF16, tag="g")
                gv = g[:, : cs * E2].rearrange("p (c e) -> p c e", e=E2)
                nc.gpsimd.dma_gather(
                    gv,
                    src_ap,
                    idx[:, c0 * 8 : (c0 + cs) * 8],
                    cs * P,
                    cs * P,
                    E2,
                )
                tv = gv[:, :, 0:D]  # T taps   [P, cs, D]
                dv = gv[:, :, D:E2]  # dT taps  [P, cs, D]
                fd = rp.tile([P, CMAX * D], BF16, tag="fd")
                fdv = fd[:, : cs * D].rearrange("p (c d) -> p c d", d=D)
                nc.vector.tensor_tensor(
                    out=fdv,
                    in0=dv,
                    in1=frep[:, c0 * D : (c0 + cs) * D].rearrange(
                        "p (c d) -> p c d", d=D
                    ),
                    op=ALU.mult,
                )
                otv = ot_all[:, c0 * D : (c0 + cs) * D].rearrange(
                    "p (c d) -> p c d", d=D
                )
                nc.vector.tensor_tensor(out=otv, in0=tv, in1=fdv, op=ALU.add)
                for gi, grp in enumerate(OUT_MERGE):
                    if grp[-1] == ci:  # last chunk of this group done
                        ga, gb = c0s[grp[0]], c0s[grp[-1]] + CHUNKS[grp[-1]]
                        out_eng = {
                            "a": nc.scalar,
                            "s": nc.sync,
                            "g": nc.gpsimd,
                        }[OUT_ENGS[gi]]
                        out_eng.dma_start(
                            out=out_d[:, ga * D : gb * D],
                            in_=ot_all[:, ga * D : gb * D],
                        )

    nc.compile()
    return nc


def _get_nc():
    global _NC
    if _NC is None:
        _NC = build_nc()
    return _NC


def _make_table(emb):
    """Pre-convolve emb onto the fine grid with the reference's exact
    normalized-Hann convention (taps r in [0,2048), |s-r|<4)."""
    s = np.arange(NFINE, dtype=np.float64) / H
    rows = np.ceil(s - 4).astype(np.int64)[:, None] + np.arange(9)
    d = s[:, None] - rows
    w = (np.cos(np.pi * d / 8) ** 2) * (np.abs(d) < 4)
    w *= (rows >= 0) & (rows < NROWS)
    T = (w[..., None] * emb[np.clip(rows, 0, NROWS - 1)].astype(np.float64)).sum(1)
    T /= w.sum(1)[:, None]
    dT = np.empty_like(T)
    dT[:-1] = T[1:] - T[:-1]
    dT[-1] = dT[-2]
    tbl = np.empty((NFINE, E2), dtype=ml_dtypes.bfloat16)
    tbl[:, :D] = T.astype(ml_dtypes.bfloat16)
    tbl[:, D:] = dT.astype(ml_dtypes.bfloat16)
    return tbl


def make_in_maps(x, embedding):
    x = np.ascontiguousarray(np.asarray(x, dtype=np.float32))
    emb = np.ascontiguousarray(np.asarray(embedding, dtype=np.float32))
    assert x.shape == (128, 200) and emb.shape == (NROWS, D)
    tbl = _make_table(emb)
    in_maps = []
    rows_per_core = x.shape[0] // NCORES
    for k in range(NCORES):
        xk = x[k * rows_per_core : (k + 1) * rows_per_core].reshape(-1)  # [3200]
        p = (xk.astype(np.float64) + 1.0) * (1024.0 * H)
        kk = np.clip(np.floor(p).astype(np.int64), 0, NFINE - 1)
        f = (p - kk).astype(ml_dtypes.bfloat16)
        idxb = kk.astype(np.int16).reshape(200, 16).T  # [16,200]; [q,t]=e(t*16+q)
        fc = f.reshape(C25, P).T  # [128,25]; [p,c]=e(c*128+p)
        meta = np.zeros((P, MCOLS), np.int16)
        meta[:, :200] = np.tile(idxb, (P // 16, 1))
        frep = np.ascontiguousarray(
            np.broadcast_to(fc[:, :, None], (P, C25, D)).reshape(P, C25 * D)
        )
        in_maps.append({"meta": meta, "frep": frep, "tbl": tbl})
    return in_maps


def unshard_out(results):
    outs = []
    for k in range(NCORES):
        o = np.asarray(results[k]["out"]).astype(np.float32)  # [128, 1600]
        o = o.reshape(P, C25, D).transpose(1, 0, 2).reshape(16, 200, D)
        outs.append(o)
    return np.ascontiguousarray(np.concatenate(outs, axis=0))


def kernel(x, embedding):
    nc = _get_nc()
    in_maps = make_in_maps(x, embedding)
    res = run_bass_kernel_spmd(nc, in_maps, list(range(NCORES)))
    return unshard_out(res.results)


if __name__ == "__main__":
    x = np.random.rand(128, 200).astype(np.float32)
    emb = np.random.randn(NROWS, D).astype(np.float32)
    out = kernel(x, emb)
    print(out.shape, out.dtype)
